# revision 14
# baseline (speedup 1.0000x reference)
"""Trainium2 Bass kernel for AdaptiveFocusedLoss, data-parallel over 8 NeuronCores.

Math (matches the jax reference exactly, up to float rounding):
  logp = log_softmax(outputs); base = -mean(logp[i, l_i])
  probs = softmax(outputs); w = W[l_i]
  mask = (c != l_i) & (w > 1) & (p > 0.2)
  penalty = sum(w*p*mask) / max(count,1) if count>0 else 0
  loss = base + 0.5 * penalty

Device-side reformulation (per core, rows sharded):
  e = exp(x)                (x = 5*randn bounded ~±30, safe in f32 without max-sub)
  s = rowsum(e), r = 1/s, p = e*r
  A  = [p > 0.2]            (bf16 0/1)
  M2 = relu(p - 0.2)        (so  p*A = M2 + 0.2*A  -> S = S_M2 + 0.2*T)
  O[i,k] = [l_i == k]       (onehot, bf16)
  PSUM accumulates (over all 128-row chunks):
     S_M2 += O^T @ M2 ; T += O^T @ A ; R += O^T @ x_bf16
  s_all kept; epilogue: lnz_sum[p] = sum_t ln(s_all[p,t])
Host side:
  ce_sum  = sum(lnz) - trace(R)            (trace(R) = sum_i x[i, l_i])
  pen_sum = <G0, S_M2 + 0.2*T>,  count = <H0, T>
  where G0 = W*(W>1) diag-zeroed, H0 = (W>1) diag-zeroed  (c != l mask == zero diag)
"""

import numpy as np

try:
    from concourse import bass, mybir, tile
    from concourse.bass_utils import run_bass_kernel_spmd
except ImportError:  # pragma: no cover
    import sys

    sys.path.insert(0, "/opt/trn_rl_repo")
    from concourse import bass, mybir, tile
    from concourse.bass_utils import run_bass_kernel_spmd

# Pin all HWDGE DMAs to a single completion lane. The DMA pseudo-instruction
# encoding has exactly one sync-wait slot; with round-robin lanes, a tile-slot
# reuse needs a cross-lane WAW wait *plus* the consumer WAR wait and walrus
# dies with "Too many sync wait commands". One lane = FIFO order makes DMA→DMA
# ordering implicit, so each DMA carries at most the consumer wait.
import concourse.tile_sem_assignment as _tsa

_tsa.NUM_HWDGE_SEMS = 1

F32 = mybir.dt.float32
BF16 = mybir.dt.bfloat16
AF = mybir.ActivationFunctionType
OP = mybir.AluOpType
AX = mybir.AxisListType

N_CORES = 8
C = 128  # num classes
B_FULL = 524288
PROB_THRESH = 0.2
CONF_PEN = 0.5
WEIGHT_THRESH = 1.0


def build_bass(rows: int, group_rows: int = 2048) -> "bass.Bass":
    """One NeuronCore's graph; SPMD across cores with different shards."""
    assert rows % group_rows == 0 and group_rows % C == 0
    ch = group_rows // C  # chunks (of 128 rows) per group
    ng = rows // group_rows  # groups
    nchunk = rows // C  # total chunks
    FD = group_rows  # free dim of the big tiles

    nc = bass.Bass()
    x_ext = nc.declare_dram_parameter("x", [rows, C], F32, isOutput=False)
    labt_ext = nc.declare_dram_parameter("labt", [C, nchunk], F32, isOutput=False)
    iota_ext = nc.declare_dram_parameter("iota", [C, C], F32, isOutput=False)
    out_ext = nc.declare_dram_parameter("out", [C, 3 * C + 1], F32, isOutput=True)

    # row = g*group_rows + p*ch + t  ->  view [p, g, t*c]; per-partition src runs
    # of ch*C*4 bytes are contiguous, so each group load is 128 big descriptors.
    x_view = x_ext[:, :].rearrange("(g p t) c -> p g (t c)", p=C, t=ch)

    with tile.TileContext(nc) as tc:
        with (
            tc.tile_pool(name="const", bufs=1) as constp,
            tc.tile_pool(name="xin", bufs=3) as xp,
            tc.tile_pool(name="ebuf", bufs=2) as ep,
            tc.tile_pool(name="pbuf", bufs=2) as pp,
            tc.tile_pool(name="ohbuf", bufs=2) as ohp,
            tc.tile_pool(name="rhsbuf", bufs=2) as rhsp,
            tc.tile_pool(name="small", bufs=2) as smallp,
            tc.tile_pool(name="psum", bufs=1, space="PSUM") as psp,
        ):
            labt = constp.tile([C, nchunk], F32)
            iota_f = constp.tile([C, C], F32)
            iota_b = constp.tile([C, C], BF16)
            s_all = constp.tile([C, nchunk], F32)
            ln_t = constp.tile([C, nchunk], F32)
            out_sb = constp.tile([C, 3 * C + 1], F32)
            acc = psp.tile([C, 3 * C], F32)

            nc.sync.dma_start(labt[:], labt_ext[:, :])
            nc.sync.dma_start(iota_f[:], iota_ext[:, :])
            nc.vector.tensor_copy(iota_b[:], iota_f[:])

            for g in range(ng):
                xt = xp.tile([C, FD], F32, tag="xt")
                et = ep.tile([C, FD], BF16, tag="et")
                pt = pp.tile([C, FD], BF16, tag="pt")
                oh = ohp.tile([C, FD], BF16, tag="oh")
                rhs = rhsp.tile([C, 3 * FD], BF16, tag="rhs")
                rt = smallp.tile([C, ch], F32, tag="rt")
                trash = smallp.tile([C, C], BF16, tag="trash")

                nc.sync.dma_start(xt[:], x_view[:, g, :])
                nc.scalar.activation(et[:], xt[:], AF.Exp)
                # row-sums s via DVE tensor_scalar + accum_out (CACHE_REDUCE):
                # runs in 4x perf mode vs tensor_reduce's 1x-only uop. The out
                # write is a throwaway bf16 copy.
                for j in range(ch):
                    t_idx = g * ch + j
                    nc.vector.tensor_scalar(
                        trash[:],
                        et[:, j * C : (j + 1) * C],
                        0.0,
                        None,
                        OP.bypass,
                        OP.add,
                        accum_out=s_all[:, t_idx : t_idx + 1],
                    )
                recip_inst = nc.vector.reciprocal(
                    rt[:], s_all[:, g * ch : (g + 1) * ch]
                )
                # p = e * (1/s), per 128-row chunk (per-partition scalar r).
                # Runs on GPSIMD (plain tensor_scalar is POOL-legal) to unload
                # the DVE, which is the bottleneck engine.
                for j in range(ch):
                    sl = slice(j * C, (j + 1) * C)
                    nc.gpsimd.tensor_scalar(
                        pt[:, sl], et[:, sl], rt[:, j : j + 1], None, OP.mult
                    )
                # rhs layout interleaved per chunk: [M2_j | A_j | x_j] so each
                # matmul streams a CONTIGUOUS N=384 (3D rhs APs halved the PE
                # column rate). rhs3[p, t, b, c], b = block.
                rhs3 = rhs[:].rearrange("p (t b c) -> p t b c", b=3, c=C)
                pt3 = pt[:].rearrange("p (t c) -> p t c", c=C)
                # A = [p > 0.2] -> block 1 ; M2 = relu(p - 0.2) -> block 0
                cpw = 4  # chunks per wide DVE op (512 free elems)
                for k in range(ch // cpw):
                    tsl = slice(k * cpw, (k + 1) * cpw)
                    nc.vector.tensor_scalar(
                        rhs3[:, tsl, 1, :],
                        pt3[:, tsl, :],
                        PROB_THRESH,
                        None,
                        OP.is_gt,
                    )
                    nc.vector.tensor_scalar(
                        rhs3[:, tsl, 0, :],
                        pt3[:, tsl, :],
                        PROB_THRESH,
                        0.0,
                        OP.subtract,
                        OP.max,
                    )
                # x cast to bf16 -> block 2 (for trace(R) = sum x[i, l_i]).
                # On ACT (not gpsimd/DVE) so xt has a single consumer engine:
                # the xt slot-reuse DMA then needs only one sync wait (the
                # DMA pseudo-instruction encoding has very few wait slots).
                nc.scalar.activation(
                    rhs3[:, :, 2, :],
                    xt[:].rearrange("p (t c) -> p t c", c=C),
                    AF.Copy,
                )
                # onehot per chunk. These only read constants, so without an
                # explicit dep Tile hoists all groups' onehots to kernel start,
                # overflowing the TensorScalarPtr sync-wait encoding. Chain
                # them into the group's DVE program order (same engine, no sem).
                for j in range(ch):
                    sl = slice(j * C, (j + 1) * C)
                    t_idx = g * ch + j
                    oh_inst = nc.vector.tensor_scalar(
                        oh[:, sl],
                        iota_b[:],
                        labt[:, t_idx : t_idx + 1],
                        None,
                        OP.is_equal,
                    )
                    tile.add_dep_helper(
                        oh_inst.ins,
                        recip_inst.ins,
                        sync=False,
                        reason="keep onehot in-group",
                    )
                # scatter-accumulate into PSUM: [S_M2 | T | R]
                for j in range(ch):
                    first = g == 0 and j == 0
                    last = g == ng - 1 and j == ch - 1
                    nc.tensor.matmul(
                        acc[:, :],
                        oh[:, j * C : (j + 1) * C],
                        rhs[:, j * 3 * C : (j + 1) * 3 * C],
                        start=first,
                        stop=last,
                    )

            # epilogue: sum of log-partition-functions, dump accumulators
            nc.scalar.activation(ln_t[:], s_all[:], AF.Ln)
            nc.vector.reduce_sum(
                out=out_sb[:, 3 * C : 3 * C + 1], in_=ln_t[:], axis=AX.X, op=OP.add
            )
            nc.vector.tensor_copy(out_sb[:, 0 : 3 * C], acc[:, :])
            nc.sync.dma_start(out_ext[:, :], out_sb[:])

    _strip_redundant_dma_lane_waits(nc)
    return nc


def _strip_redundant_dma_lane_waits(nc):
    """Every TPB instruction encoding holds exactly ONE sync-wait slot; walrus
    raises "Too many sync wait commands" on the rest. Legalize:

    - InstDMACopy: with all HWDGE DMAs pinned to one completion lane
      (NUM_HWDGE_SEMS=1 above), the lane-predecessor wait Tile embeds is
      redundant whenever the DMA also carries an engine (consumer WAR) wait:
      the consumer's read implies the previous slot-write completed, and
      same-address descriptor streams execute FIFO per SDMA engine. Drop
      lane waits.
    - Engine instructions (matmul, activation, drain, ...): keep one wait
      embedded; hoist the rest into standalone InstEventSemaphore waits on
      the same engine queue immediately before the instruction.
    """
    f = nc.m.functions[0]
    for blk in list(f.blocks):
        insts = list(blk.instructions)
        new_insts = []
        changed = False
        for inst in insts:
            si = inst.sync_info
            waits = list(si.on_wait) if (si and si.on_wait) else []
            if len(waits) > 1:
                changed = True
                if type(inst).__name__ == "InstDMACopy":
                    keep = [
                        w
                        for w in waits
                        if not w.ant_name.startswith(("DMAHW", "DMASW"))
                    ] or waits[-1:]
                else:
                    keep = waits
                if len(keep) > 1:
                    extra, keep = keep[:-1], keep[-1:]
                    for k, w in enumerate(extra):
                        es = mybir.InstEventSemaphore(
                            name=f"{inst.name}-wsplit{k}",
                            engine=inst.engine,
                            ins=[],
                            outs=[],
                            sync_info=mybir.SyncInfo(on_wait=[w], on_update=[]),
                        )
                        nc.register_instruction(es)
                        new_insts.append(es)
                si.on_wait = keep
            new_insts.append(inst)
        if changed:
            blk.instructions = new_insts


def _shard_inputs(outputs: np.ndarray, labels: np.ndarray, rows: int, group_rows: int):
    """Build per-core in_maps. Row mapping inside a core/group: row = g*G + p*ch + t."""
    ch = group_rows // C
    ng = rows // group_rows
    iota = np.tile(np.arange(C, dtype=np.float32), (C, 1))
    in_maps = []
    n_cores = outputs.shape[0] // rows
    for i in range(n_cores):
        lab_i = labels[i * rows : (i + 1) * rows].astype(np.float32)
        # labt[p, g*ch + t] = labels[g*group_rows + p*ch + t]
        labt = np.ascontiguousarray(
            lab_i.reshape(ng, C, ch).transpose(1, 0, 2).reshape(C, ng * ch)
        )
        in_maps.append(
            {
                "x": np.ascontiguousarray(outputs[i * rows : (i + 1) * rows]),
                "labt": labt,
                "iota": iota,
            }
        )
    return in_maps


def combine_outputs(core_outs, lnz_extra=None, confusion_weights=None, B=None):
    """Host-side reduction of per-core [128, 385] partials -> scalar loss."""
    S_M2 = np.zeros((C, C), np.float64)
    T = np.zeros((C, C), np.float64)
    R = np.zeros((C, C), np.float64)
    lnz_sum = 0.0
    for o in core_outs:
        o = np.asarray(o, np.float64)
        S_M2 += o[:, 0:C]
        T += o[:, C : 2 * C]
        R += o[:, 2 * C : 3 * C]
        lnz_sum += o[:, 3 * C].sum()
    ce_sum = lnz_sum - np.trace(R)
    base = ce_sum / B

    W = np.asarray(confusion_weights, np.float64)
    wmask = W > WEIGHT_THRESH
    G0 = np.where(wmask, W, 0.0)
    np.fill_diagonal(G0, 0.0)
    H0 = wmask.astype(np.float64)
    np.fill_diagonal(H0, 0.0)

    S = S_M2 + PROB_THRESH * T
    pen_sum = float((G0 * S).sum())
    count = float(np.rint((H0 * T).sum()))
    penalty = pen_sum / max(count, 1.0) if count > 0 else 0.0
    return np.float32(base + CONF_PEN * penalty)


_CACHE = {}


def _get_nc(rows: int, group_rows: int):
    key = (rows, group_rows)
    if key not in _CACHE:
        _CACHE[key] = build_bass(rows, group_rows)
    return _CACHE[key]


def kernel(outputs: np.ndarray, labels: np.ndarray, confusion_weights: np.ndarray, **kw):
    outputs = np.asarray(outputs, np.float32)
    labels = np.asarray(labels)
    B = outputs.shape[0]
    rows = B // N_CORES
    group_rows = 2048
    nc = _get_nc(rows, group_rows)
    in_maps = _shard_inputs(outputs, labels, rows, group_rows)
    res = run_bass_kernel_spmd(nc, in_maps, core_ids=list(range(N_CORES)))
    core_outs = [r["out"] for r in res.results]
    return combine_outputs(core_outs, confusion_weights=confusion_weights, B=B)


if __name__ == "__main__":
    # smoke test on random data (host-side check only builds the graph)
    nc = build_bass(4096, 2048)
    print("built ok:", nc)


# revision 21
# speedup vs baseline: 3.5959x; 3.5959x over previous
"""Trainium2 Bass kernel for AdaptiveFocusedLoss, data-parallel over 8 NeuronCores.

Math (matches the jax reference exactly, up to float rounding):
  logp = log_softmax(outputs); base = -mean(logp[i, l_i])
  probs = softmax(outputs); w = W[l_i]
  mask = (c != l_i) & (w > 1) & (p > 0.2)
  penalty = sum(w*p*mask) / max(count,1) if count>0 else 0
  loss = base + 0.5 * penalty

Device-side reformulation (per core, rows sharded):
  e = exp(x)                (x = 5*randn bounded ~±30, safe in f32 without max-sub)
  s = rowsum(e), r = 1/s, p = e*r
  A  = [p > 0.2]            (bf16 0/1)
  M2 = relu(p - 0.2)        (so  p*A = M2 + 0.2*A  -> S = S_M2 + 0.2*T)
  O[i,k] = [l_i == k]       (onehot, bf16)
  PSUM accumulates (over all 128-row chunks):
     S_M2 += O^T @ M2 ; T += O^T @ A ; R += O^T @ x_bf16
  s_all kept; epilogue: lnz_sum[p] = sum_t ln(s_all[p,t])
Host side:
  ce_sum  = sum(lnz) - trace(R)            (trace(R) = sum_i x[i, l_i])
  pen_sum = <G0, S_M2 + 0.2*T>,  count = <H0, T>
  where G0 = W*(W>1) diag-zeroed, H0 = (W>1) diag-zeroed  (c != l mask == zero diag)
"""

import numpy as np

try:
    from concourse import bass, mybir, tile
    from concourse.bass_utils import run_bass_kernel_spmd
except ImportError:  # pragma: no cover
    import sys

    sys.path.insert(0, "/opt/trn_rl_repo")
    from concourse import bass, mybir, tile
    from concourse.bass_utils import run_bass_kernel_spmd

# Pin all HWDGE DMAs to a single completion lane. The DMA pseudo-instruction
# encoding has exactly one sync-wait slot; with round-robin lanes, a tile-slot
# reuse needs a cross-lane WAW wait *plus* the consumer WAR wait and walrus
# dies with "Too many sync wait commands". One lane = FIFO order makes DMA→DMA
# ordering implicit, so each DMA carries at most the consumer wait.
import concourse.tile_sem_assignment as _tsa

_tsa.NUM_HWDGE_SEMS = 1

F32 = mybir.dt.float32
BF16 = mybir.dt.bfloat16
AF = mybir.ActivationFunctionType
OP = mybir.AluOpType
AX = mybir.AxisListType

N_CORES = 8
C = 128  # num classes
B_FULL = 524288
PROB_THRESH = 0.2
CONF_PEN = 0.5
WEIGHT_THRESH = 1.0


def build_bass(rows: int, group_rows: int = 2048) -> "bass.Bass":
    """One NeuronCore's graph; SPMD across cores with different shards."""
    assert rows % group_rows == 0 and group_rows % C == 0
    ch = group_rows // C  # chunks (of 128 rows) per group
    ng = rows // group_rows  # groups
    nchunk = rows // C  # total chunks
    FD = group_rows  # free dim of the big tiles

    nc = bass.Bass()
    x_ext = nc.declare_dram_parameter("x", [rows, C], BF16, isOutput=False)
    oh_ext = nc.declare_dram_parameter("oh", [C, nchunk * C], BF16, isOutput=False)
    out_ext = nc.declare_dram_parameter("out", [C, 3 * C + 1], F32, isOutput=True)

    # row = g*group_rows + p*ch + t  ->  view [p, g, t, c]; per-partition src
    # runs of ch*C*2 bytes are contiguous, so each group load is 128 big
    # descriptors.
    x_view = x_ext[:, :].rearrange("(g p t) c -> p g t c", p=C, t=ch)

    with tile.TileContext(nc) as tc:
        with (
            tc.tile_pool(name="const", bufs=1) as constp,
            tc.tile_pool(name="ebuf", bufs=2) as ep,
            tc.tile_pool(name="pbuf", bufs=2) as pp,
            tc.tile_pool(name="ohbuf", bufs=3) as ohp,
            tc.tile_pool(name="rhsbuf", bufs=2) as rhsp,
            tc.tile_pool(name="small", bufs=2) as smallp,
            tc.tile_pool(name="psum", bufs=1, space="PSUM") as psp,
        ):
            s_all = constp.tile([C, nchunk], F32)
            ln_t = constp.tile([C, nchunk], F32)
            out_sb = constp.tile([C, 3 * C + 1], F32)
            nthr = constp.tile([C, 1], F32)  # -PROB_THRESH bias for ACT Relu
            acc = psp.tile([C, 3 * C], F32)
            nc.vector.memset(nthr[:], -PROB_THRESH)

            for g in range(ng):
                et = ep.tile([C, FD], BF16, tag="et")
                pt = pp.tile([C, FD], BF16, tag="pt")
                oh = ohp.tile([C, FD], BF16, tag="oh")
                rhs = rhsp.tile([C, 3 * FD], BF16, tag="rhs")
                rt = smallp.tile([C, ch], F32, tag="rt")
                trash = smallp.tile([C, C], BF16, tag="trash")

                # rhs interleaved per chunk: [M2_j | A_j | x_j]; each matmul
                # streams a CONTIGUOUS N=384. x is DMA'd straight into block 2.
                rhs3 = rhs[:].rearrange("p (t b c) -> p t b c", b=3, c=C)
                nc.sync.dma_start(rhs3[:, :, 2, :], x_view[:, g, :, :])
                nc.sync.dma_start(oh[:], oh_ext[:, g * FD : (g + 1) * FD])
                nc.scalar.activation(et[:], rhs3[:, :, 2, :], AF.Exp)
                # row-sums s (DVE tensor_reduce; per-chunk stays under DRAIN)
                for j in range(ch):
                    t_idx = g * ch + j
                    nc.vector.reduce_sum(
                        out=s_all[:, t_idx : t_idx + 1],
                        in_=et[:, j * C : (j + 1) * C],
                        axis=AX.X,
                    )
                nc.vector.reciprocal(rt[:], s_all[:, g * ch : (g + 1) * ch])
                # p = e * (1/s), per 128-row chunk (per-partition scalar r)
                for j in range(ch):
                    sl = slice(j * C, (j + 1) * C)
                    nc.vector.tensor_scalar(
                        pt[:, sl], et[:, sl], rt[:, j : j + 1], None, OP.mult
                    )
                pt3 = pt[:].rearrange("p (t c) -> p t c", c=C)
                # A = [p > 0.2] -> block 1 (DVE, wide imm ops)
                cpw = 4  # chunks per wide DVE op (512 free elems)
                for k in range(ch // cpw):
                    tsl = slice(k * cpw, (k + 1) * cpw)
                    nc.vector.tensor_scalar(
                        rhs3[:, tsl, 1, :],
                        pt3[:, tsl, :],
                        PROB_THRESH,
                        None,
                        OP.is_gt,
                    )
                # M2 = relu(p - 0.2) -> block 0 (on ACT, one op per group)
                nc.scalar.activation(
                    rhs3[:, :, 0, :], pt3[:, :, :], AF.Relu, bias=nthr[:, 0:1]
                )
                # scatter-accumulate into PSUM: [S_M2 | T | R]
                for j in range(ch):
                    first = g == 0 and j == 0
                    last = g == ng - 1 and j == ch - 1
                    nc.tensor.matmul(
                        acc[:, :],
                        oh[:, j * C : (j + 1) * C],
                        rhs[:, j * 3 * C : (j + 1) * 3 * C],
                        start=first,
                        stop=last,
                    )

            # epilogue: sum of log-partition-functions, dump accumulators
            nc.scalar.activation(ln_t[:], s_all[:], AF.Ln)
            nc.vector.reduce_sum(
                out=out_sb[:, 3 * C : 3 * C + 1], in_=ln_t[:], axis=AX.X, op=OP.add
            )
            nc.vector.tensor_copy(out_sb[:, 0 : 3 * C], acc[:, :])
            nc.sync.dma_start(out_ext[:, :], out_sb[:])

    _strip_redundant_dma_lane_waits(nc)
    return nc


def _strip_redundant_dma_lane_waits(nc):
    """Every TPB instruction encoding holds exactly ONE sync-wait slot; walrus
    raises "Too many sync wait commands" on the rest. Legalize:

    - InstDMACopy: with all HWDGE DMAs pinned to one completion lane
      (NUM_HWDGE_SEMS=1 above), the lane-predecessor wait Tile embeds is
      redundant whenever the DMA also carries an engine (consumer WAR) wait:
      the consumer's read implies the previous slot-write completed, and
      same-address descriptor streams execute FIFO per SDMA engine. Drop
      lane waits.
    - Engine instructions (matmul, activation, drain, ...): keep one wait
      embedded; hoist the rest into standalone InstEventSemaphore waits on
      the same engine queue immediately before the instruction.
    """
    f = nc.m.functions[0]
    for blk in list(f.blocks):
        insts = list(blk.instructions)
        new_insts = []
        changed = False
        for inst in insts:
            si = inst.sync_info
            waits = list(si.on_wait) if (si and si.on_wait) else []
            if len(waits) > 1:
                changed = True
                if type(inst).__name__ == "InstDMACopy":
                    keep = [
                        w
                        for w in waits
                        if not w.ant_name.startswith(("DMAHW", "DMASW"))
                    ] or waits[-1:]
                else:
                    keep = waits
                if len(keep) > 1:
                    extra, keep = keep[:-1], keep[-1:]
                    for k, w in enumerate(extra):
                        es = mybir.InstEventSemaphore(
                            name=f"{inst.name}-wsplit{k}",
                            engine=inst.engine,
                            ins=[],
                            outs=[],
                            sync_info=mybir.SyncInfo(on_wait=[w], on_update=[]),
                        )
                        nc.register_instruction(es)
                        new_insts.append(es)
                si.on_wait = keep
            new_insts.append(inst)
        if changed:
            blk.instructions = new_insts


def _shard_inputs(outputs: np.ndarray, labels: np.ndarray, rows: int, group_rows: int):
    """Build per-core in_maps. Row mapping inside a core/group: row = g*G + p*ch + t."""
    import ml_dtypes

    bf16 = ml_dtypes.bfloat16
    ch = group_rows // C
    ng = rows // group_rows
    in_maps = []
    n_cores = outputs.shape[0] // rows
    cls = np.arange(C, dtype=np.int32)
    for i in range(n_cores):
        lab_i = labels[i * rows : (i + 1) * rows].astype(np.int32)
        # oh[p, ((g*ch + t)*C + c)] = 1.0 if labels[g*G + p*ch + t] == c
        # labT[p, g, t] = labels[g*G + p*ch + t]
        labT = lab_i.reshape(ng, C, ch).transpose(1, 0, 2)  # [C, ng, ch]
        oh = (labT[:, :, :, None] == cls[None, None, None, :]).astype(bf16)
        in_maps.append(
            {
                "x": np.ascontiguousarray(outputs[i * rows : (i + 1) * rows]).astype(
                    bf16
                ),
                "oh": np.ascontiguousarray(oh.reshape(C, ng * ch * C)),
            }
        )
    return in_maps


def combine_outputs(core_outs, lnz_extra=None, confusion_weights=None, B=None):
    """Host-side reduction of per-core [128, 385] partials -> scalar loss."""
    S_M2 = np.zeros((C, C), np.float64)
    T = np.zeros((C, C), np.float64)
    R = np.zeros((C, C), np.float64)
    lnz_sum = 0.0
    for o in core_outs:
        o = np.asarray(o, np.float64)
        S_M2 += o[:, 0:C]
        T += o[:, C : 2 * C]
        R += o[:, 2 * C : 3 * C]
        lnz_sum += o[:, 3 * C].sum()
    ce_sum = lnz_sum - np.trace(R)
    base = ce_sum / B

    W = np.asarray(confusion_weights, np.float64)
    wmask = W > WEIGHT_THRESH
    G0 = np.where(wmask, W, 0.0)
    np.fill_diagonal(G0, 0.0)
    H0 = wmask.astype(np.float64)
    np.fill_diagonal(H0, 0.0)

    S = S_M2 + PROB_THRESH * T
    pen_sum = float((G0 * S).sum())
    count = float(np.rint((H0 * T).sum()))
    penalty = pen_sum / max(count, 1.0) if count > 0 else 0.0
    return np.float32(base + CONF_PEN * penalty)


_CACHE = {}


def _get_nc(rows: int, group_rows: int):
    key = (rows, group_rows)
    if key not in _CACHE:
        _CACHE[key] = build_bass(rows, group_rows)
    return _CACHE[key]


def kernel(outputs: np.ndarray, labels: np.ndarray, confusion_weights: np.ndarray, **kw):
    outputs = np.asarray(outputs, np.float32)
    labels = np.asarray(labels)
    B = outputs.shape[0]
    rows = B // N_CORES
    group_rows = 2048
    nc = _get_nc(rows, group_rows)
    in_maps = _shard_inputs(outputs, labels, rows, group_rows)
    res = run_bass_kernel_spmd(nc, in_maps, core_ids=list(range(N_CORES)))
    core_outs = [r["out"] for r in res.results]
    return combine_outputs(core_outs, confusion_weights=confusion_weights, B=B)


if __name__ == "__main__":
    # smoke test on random data (host-side check only builds the graph)
    nc = build_bass(4096, 2048)
    print("built ok:", nc)


# revision 23
# speedup vs baseline: 5.3071x; 1.4759x over previous
"""Trainium2 Bass kernel for AdaptiveFocusedLoss, data-parallel over 8 NeuronCores.

Math (matches the jax reference exactly, up to float rounding):
  logp = log_softmax(outputs); base = -mean(logp[i, l_i])
  probs = softmax(outputs); w = W[l_i]
  mask = (c != l_i) & (w > 1) & (p > 0.2)
  penalty = sum(w*p*mask) / max(count,1) if count>0 else 0
  loss = base + 0.5 * penalty

Device-side reformulation (per core, rows sharded):
  e = exp(x)                (x = 5*randn bounded ~±30, safe in f32 without max-sub)
  s = rowsum(e), r = 1/s, p = e*r
  A  = [p > 0.2]            (bf16 0/1)
  M2 = relu(p - 0.2)        (so  p*A = M2 + 0.2*A  -> S = S_M2 + 0.2*T)
  O[i,k] = [l_i == k]       (onehot, bf16)
  PSUM accumulates (over all 128-row chunks):
     S_M2 += O^T @ M2 ; T += O^T @ A ; R += O^T @ x_bf16
  s_all kept; epilogue: lnz_sum[p] = sum_t ln(s_all[p,t])
Host side:
  ce_sum  = sum(lnz) - trace(R)            (trace(R) = sum_i x[i, l_i])
  pen_sum = <G0, S_M2 + 0.2*T>,  count = <H0, T>
  where G0 = W*(W>1) diag-zeroed, H0 = (W>1) diag-zeroed  (c != l mask == zero diag)
"""

import numpy as np

try:
    from concourse import bass, mybir, tile
    from concourse.bass_utils import run_bass_kernel_spmd
except ImportError:  # pragma: no cover
    import sys

    sys.path.insert(0, "/opt/trn_rl_repo")
    from concourse import bass, mybir, tile
    from concourse.bass_utils import run_bass_kernel_spmd

# Pin all HWDGE DMAs to a single completion lane. The DMA pseudo-instruction
# encoding has exactly one sync-wait slot; with round-robin lanes, a tile-slot
# reuse needs a cross-lane WAW wait *plus* the consumer WAR wait and walrus
# dies with "Too many sync wait commands". One lane = FIFO order makes DMA→DMA
# ordering implicit, so each DMA carries at most the consumer wait.
import concourse.tile_sem_assignment as _tsa

_tsa.NUM_HWDGE_SEMS = 1

F32 = mybir.dt.float32
BF16 = mybir.dt.bfloat16
AF = mybir.ActivationFunctionType
OP = mybir.AluOpType
AX = mybir.AxisListType

N_CORES = 8
C = 128  # num classes
B_FULL = 524288
PROB_THRESH = 0.2
CONF_PEN = 0.5
WEIGHT_THRESH = 1.0


def build_bass(rows: int, group_rows: int = 2048) -> "bass.Bass":
    """One NeuronCore's graph; SPMD across cores with different shards."""
    assert rows % group_rows == 0 and group_rows % C == 0
    ch = group_rows // C  # chunks (of 128 rows) per group
    ng = rows // group_rows  # groups
    nchunk = rows // C  # total chunks
    FD = group_rows  # free dim of the big tiles

    nc = bass.Bass()
    x_ext = nc.declare_dram_parameter("x", [rows, C], BF16, isOutput=False)
    oh_ext = nc.declare_dram_parameter("oh", [C, nchunk * C], BF16, isOutput=False)
    out_ext = nc.declare_dram_parameter("out", [C, 3 * C + 1], F32, isOutput=True)

    # row = g*group_rows + p*ch + t  ->  view [p, g, t, c]; per-partition src
    # runs of ch*C*2 bytes are contiguous, so each group load is 128 big
    # descriptors.
    x_view = x_ext[:, :].rearrange("(g p t) c -> p g t c", p=C, t=ch)

    with tile.TileContext(nc) as tc:
        with (
            tc.tile_pool(name="const", bufs=1) as constp,
            tc.tile_pool(name="ebuf", bufs=3) as ep,
            tc.tile_pool(name="pbuf", bufs=3) as pp,
            tc.tile_pool(name="ohbuf", bufs=3) as ohp,
            tc.tile_pool(name="rhsbuf", bufs=3) as rhsp,
            tc.tile_pool(name="small", bufs=4) as smallp,
            tc.tile_pool(name="psum", bufs=1, space="PSUM") as psp,
        ):
            s_all = constp.tile([C, nchunk], F32)
            ln_t = constp.tile([C, nchunk], F32)
            out_sb = constp.tile([C, 3 * C + 1], F32)
            nthr = constp.tile([C, 1], F32)  # -PROB_THRESH bias for ACT Relu
            acc = psp.tile([C, 3 * C], F32)
            nc.vector.memset(nthr[:], -PROB_THRESH)

            for g in range(ng):
                et = ep.tile([C, FD], BF16, tag="et")
                pt = pp.tile([C, FD], BF16, tag="pt")
                oh = ohp.tile([C, FD], BF16, tag="oh")
                rhs = rhsp.tile([C, 3 * FD], BF16, tag="rhs")
                rt = smallp.tile([C, ch], F32, tag="rt")

                # rhs interleaved per chunk: [M2_j | A_j | x_j]; each matmul
                # streams a CONTIGUOUS N=384. x is DMA'd straight into block 2.
                rhs3 = rhs[:].rearrange("p (t b c) -> p t b c", b=3, c=C)
                nc.sync.dma_start(rhs3[:, :, 2, :], x_view[:, g, :, :])
                nc.sync.dma_start(oh[:], oh_ext[:, g * FD : (g + 1) * FD])
                nc.scalar.activation(et[:], rhs3[:, :, 2, :], AF.Exp)
                pt3 = pt[:].rearrange("p (t c) -> p t c", c=C)
                # 4-chunk sub-blocks: [4 row-sum reduces, reciprocal, 4 p-mults,
                # A-threshold] — the split reciprocal shortens the dependency
                # chain so p of sub-block k starts before reduces of k+1 finish
                cpw = 4
                for k in range(ch // cpw):
                    tsl = slice(k * cpw, (k + 1) * cpw)
                    for j in range(k * cpw, (k + 1) * cpw):
                        t_idx = g * ch + j
                        nc.vector.reduce_sum(
                            out=s_all[:, t_idx : t_idx + 1],
                            in_=et[:, j * C : (j + 1) * C],
                            axis=AX.X,
                        )
                    nc.vector.reciprocal(
                        rt[:, tsl],
                        s_all[:, g * ch + k * cpw : g * ch + (k + 1) * cpw],
                    )
                    # p = e * (1/s), per 128-row chunk (per-partition scalar r)
                    for j in range(k * cpw, (k + 1) * cpw):
                        sl = slice(j * C, (j + 1) * C)
                        nc.vector.tensor_scalar(
                            pt[:, sl], et[:, sl], rt[:, j : j + 1], None, OP.mult
                        )
                    # A = [p > 0.2] -> block 1 (DVE, wide imm op)
                    nc.vector.tensor_scalar(
                        rhs3[:, tsl, 1, :],
                        pt3[:, tsl, :],
                        PROB_THRESH,
                        None,
                        OP.is_gt,
                    )
                    # M2 = relu(p - 0.2) -> block 0 (ACT), half-group ops
                    if k % 2 == 1:
                        hsl = slice((k - 1) * cpw, (k + 1) * cpw)
                        nc.scalar.activation(
                            rhs3[:, hsl, 0, :],
                            pt3[:, hsl, :],
                            AF.Relu,
                            bias=nthr[:, 0:1],
                        )
                # scatter-accumulate into PSUM: [S_M2 | T | R]
                for j in range(ch):
                    first = g == 0 and j == 0
                    last = g == ng - 1 and j == ch - 1
                    nc.tensor.matmul(
                        acc[:, :],
                        oh[:, j * C : (j + 1) * C],
                        rhs[:, j * 3 * C : (j + 1) * 3 * C],
                        start=first,
                        stop=last,
                    )

            # epilogue: sum of log-partition-functions, dump accumulators
            nc.scalar.activation(ln_t[:], s_all[:], AF.Ln)
            nc.vector.reduce_sum(
                out=out_sb[:, 3 * C : 3 * C + 1], in_=ln_t[:], axis=AX.X, op=OP.add
            )
            nc.vector.tensor_copy(out_sb[:, 0 : 3 * C], acc[:, :])
            nc.sync.dma_start(out_ext[:, :], out_sb[:])

    _strip_redundant_dma_lane_waits(nc)
    return nc


def _strip_redundant_dma_lane_waits(nc):
    """Every TPB instruction encoding holds exactly ONE sync-wait slot; walrus
    raises "Too many sync wait commands" on the rest. Legalize:

    - InstDMACopy: with all HWDGE DMAs pinned to one completion lane
      (NUM_HWDGE_SEMS=1 above), the lane-predecessor wait Tile embeds is
      redundant whenever the DMA also carries an engine (consumer WAR) wait:
      the consumer's read implies the previous slot-write completed, and
      same-address descriptor streams execute FIFO per SDMA engine. Drop
      lane waits.
    - Engine instructions (matmul, activation, drain, ...): keep one wait
      embedded; hoist the rest into standalone InstEventSemaphore waits on
      the same engine queue immediately before the instruction.
    """
    f = nc.m.functions[0]
    for blk in list(f.blocks):
        insts = list(blk.instructions)
        new_insts = []
        changed = False
        for inst in insts:
            si = inst.sync_info
            waits = list(si.on_wait) if (si and si.on_wait) else []
            if len(waits) > 1:
                changed = True
                if type(inst).__name__ == "InstDMACopy":
                    keep = [
                        w
                        for w in waits
                        if not w.ant_name.startswith(("DMAHW", "DMASW"))
                    ] or waits[-1:]
                else:
                    keep = waits
                if len(keep) > 1:
                    extra, keep = keep[:-1], keep[-1:]
                    for k, w in enumerate(extra):
                        es = mybir.InstEventSemaphore(
                            name=f"{inst.name}-wsplit{k}",
                            engine=inst.engine,
                            ins=[],
                            outs=[],
                            sync_info=mybir.SyncInfo(on_wait=[w], on_update=[]),
                        )
                        nc.register_instruction(es)
                        new_insts.append(es)
                si.on_wait = keep
            new_insts.append(inst)
        if changed:
            blk.instructions = new_insts


def _shard_inputs(outputs: np.ndarray, labels: np.ndarray, rows: int, group_rows: int):
    """Build per-core in_maps. Row mapping inside a core/group: row = g*G + p*ch + t."""
    import ml_dtypes

    bf16 = ml_dtypes.bfloat16
    ch = group_rows // C
    ng = rows // group_rows
    in_maps = []
    n_cores = outputs.shape[0] // rows
    cls = np.arange(C, dtype=np.int32)
    for i in range(n_cores):
        lab_i = labels[i * rows : (i + 1) * rows].astype(np.int32)
        # oh[p, ((g*ch + t)*C + c)] = 1.0 if labels[g*G + p*ch + t] == c
        # labT[p, g, t] = labels[g*G + p*ch + t]
        labT = lab_i.reshape(ng, C, ch).transpose(1, 0, 2)  # [C, ng, ch]
        oh = (labT[:, :, :, None] == cls[None, None, None, :]).astype(bf16)
        in_maps.append(
            {
                "x": np.ascontiguousarray(outputs[i * rows : (i + 1) * rows]).astype(
                    bf16
                ),
                "oh": np.ascontiguousarray(oh.reshape(C, ng * ch * C)),
            }
        )
    return in_maps


def combine_outputs(core_outs, lnz_extra=None, confusion_weights=None, B=None):
    """Host-side reduction of per-core [128, 385] partials -> scalar loss."""
    S_M2 = np.zeros((C, C), np.float64)
    T = np.zeros((C, C), np.float64)
    R = np.zeros((C, C), np.float64)
    lnz_sum = 0.0
    for o in core_outs:
        o = np.asarray(o, np.float64)
        S_M2 += o[:, 0:C]
        T += o[:, C : 2 * C]
        R += o[:, 2 * C : 3 * C]
        lnz_sum += o[:, 3 * C].sum()
    ce_sum = lnz_sum - np.trace(R)
    base = ce_sum / B

    W = np.asarray(confusion_weights, np.float64)
    wmask = W > WEIGHT_THRESH
    G0 = np.where(wmask, W, 0.0)
    np.fill_diagonal(G0, 0.0)
    H0 = wmask.astype(np.float64)
    np.fill_diagonal(H0, 0.0)

    S = S_M2 + PROB_THRESH * T
    pen_sum = float((G0 * S).sum())
    count = float(np.rint((H0 * T).sum()))
    penalty = pen_sum / max(count, 1.0) if count > 0 else 0.0
    return np.float32(base + CONF_PEN * penalty)


_CACHE = {}


def _get_nc(rows: int, group_rows: int):
    key = (rows, group_rows)
    if key not in _CACHE:
        _CACHE[key] = build_bass(rows, group_rows)
    return _CACHE[key]


def kernel(outputs: np.ndarray, labels: np.ndarray, confusion_weights: np.ndarray, **kw):
    outputs = np.asarray(outputs, np.float32)
    labels = np.asarray(labels)
    B = outputs.shape[0]
    rows = B // N_CORES
    group_rows = 2048
    nc = _get_nc(rows, group_rows)
    in_maps = _shard_inputs(outputs, labels, rows, group_rows)
    res = run_bass_kernel_spmd(nc, in_maps, core_ids=list(range(N_CORES)))
    core_outs = [r["out"] for r in res.results]
    return combine_outputs(core_outs, confusion_weights=confusion_weights, B=B)


if __name__ == "__main__":
    # smoke test on random data (host-side check only builds the graph)
    nc = build_bass(4096, 2048)
    print("built ok:", nc)
